# revision 9
# baseline (speedup 1.0000x reference)
"""3x3 median filter (zero-padded) on TRN2, 8 NeuronCores, exact fp32.

Input  x: (32, 3, 512, 512) float32
Output  : (32, 3, 512, 512) float32, bit-exact vs jnp sort-based reference.

Strategy
--------
Pure data parallel: batch dim sharded 4-per-core across 8 cores. Per core the
12 images (4 batch x 3 chan) are processed in 6 pairs x 2 vertical halves.

Exact fp32 median-of-9 via the classic column-sort decomposition with
pair-sharing, 15 min/max tensor ops per output element, all on the DVE:

  stage 1 (vertical, 5 ops/elem): rows laid out as even/odd row-pair tiles
    E[p]=row r0+2p, O[p]=row r0+2p+1 plus shifted loads E_sh (+1 row-pair
    element) and O_sh (-1). pair min/max shared by the two windows centered
    on the pair's rows; third element closes each sort3 -> per-column
    (min, med, max) fields for even-row and odd-row outputs.

  stage 2 (horizontal, 10 ops/elem): zero-padded width-514 fields; column
    pair-sharing at even columns; med9 = med3(max3(mins), med3(meds),
    min3(maxes)).

All shifts along W are strided free-dim APs (measured: same DVE cost as
dense). Vertical halo comes from the extra strided HBM loads (reads x2,
hidden under compute). Zero padding handled by memset pad columns / halo
partitions, so every op is a full-width partition-aligned tensor op.
"""
import sys

if "/opt/trn_rl_repo" not in sys.path:
    sys.path.insert(0, "/opt/trn_rl_repo")

import numpy as np
import concourse.bacc as bacc
import concourse.mybir as mybir
import concourse.tile as tile
from concourse import bass_utils

B, C, H, W = 32, 3, 512, 512
N_CORES = 8
B_PER = B // N_CORES          # 4 batches per core
NIMG = B_PER * C              # 12 images per core
GIMG = 4                      # images per tile group
FW = GIMG * W                 # free width of row tiles (1024)
PW = W + 2                    # padded per-image width (514)
FP = GIMG * PW                # free width of padded tiles (1028)
HW_half = H // 2              # 256 rows per vertical half
P = 128                       # partitions = row pairs per half

F32 = mybir.dt.float32
MIN = mybir.AluOpType.min
MAX = mybir.AluOpType.max

_PROGRAM = None


def _stage2(nc, pm, PMN, PMD, PMX, OUT, rowtag):
    """Horizontal pass: padded (min, med, max) fields [128, GIMG*514] ->
    median written into OUT [128, GIMG*512] (interleaved columns)."""
    v = lambda T: T[:].rearrange("p (i w) -> p i w", w=PW)   # [128, GIMG, 514]
    ov = OUT[:].rearrange("p (i w) -> p i w", w=W)            # [128, GIMG, 512]
    mn, md, mx = v(PMN), v(PMD), v(PMX)

    def t2(tag, fw=W):
        return pm.tile([P, GIMG * fw], F32, tag=tag, name=tag)

    def tv(T, fw):
        return T[:].rearrange("p (i w) -> p i w", w=fw)

    NP = PW // 2  # 257 pairs per image
    U = t2("U", NP); Vt = t2("V", NP); Qmn = t2("Qmn", NP); Qmx = t2("Qmx", NP)
    Uv, Vv, Qmnv, Qmxv = tv(U, NP), tv(Vt, NP), tv(Qmn, NP), tv(Qmx, NP)
    HWW = W // 2  # 256 outputs per column parity

    # pairs over padded columns (2k, 2k+1)
    nc.vector.tensor_tensor(Uv, mn[:, :, 0:PW:2], mn[:, :, 1:PW:2], op=MAX)
    nc.vector.tensor_tensor(Vv, mx[:, :, 0:PW:2], mx[:, :, 1:PW:2], op=MIN)
    nc.vector.tensor_tensor(Qmnv, md[:, :, 0:PW:2], md[:, :, 1:PW:2], op=MIN)
    nc.vector.tensor_tensor(Qmxv, md[:, :, 0:PW:2], md[:, :, 1:PW:2], op=MAX)

    A_e = t2("A_e", HWW); A_o = t2("A_o", HWW)
    C_e = t2("C_e", HWW); C_o = t2("C_o", HWW)
    t_be = t2("t_be", HWW); B_e = t2("B_e", HWW)
    t_bo = t2("t_bo", HWW); B_o = t2("B_o", HWW)
    Aev, Aov = tv(A_e, HWW), tv(A_o, HWW)
    Cev, Cov = tv(C_e, HWW), tv(C_o, HWW)
    tbev, Bev = tv(t_be, HWW), tv(B_e, HWW)
    tbov, Bov = tv(t_bo, HWW), tv(B_o, HWW)

    # even output columns w=2m: pair k=m + third padded col 2m+2
    nc.vector.tensor_tensor(Aev, Uv[:, :, 0:HWW], mn[:, :, 2:PW:2], op=MAX)
    nc.vector.tensor_tensor(Cev, Vv[:, :, 0:HWW], mx[:, :, 2:PW:2], op=MIN)
    nc.vector.tensor_tensor(tbev, Qmxv[:, :, 0:HWW], md[:, :, 2:PW:2], op=MIN)
    nc.vector.tensor_tensor(Bev, Qmnv[:, :, 0:HWW], tbev, op=MAX)
    # odd output columns w=2m+1: pair k=m+1 + third padded col 2m+1
    nc.vector.tensor_tensor(Aov, Uv[:, :, 1 : HWW + 1], mn[:, :, 1 : PW - 1 : 2], op=MAX)
    nc.vector.tensor_tensor(Cov, Vv[:, :, 1 : HWW + 1], mx[:, :, 1 : PW - 1 : 2], op=MIN)
    nc.vector.tensor_tensor(tbov, Qmxv[:, :, 1 : HWW + 1], md[:, :, 1 : PW - 1 : 2], op=MIN)
    nc.vector.tensor_tensor(Bov, Qmnv[:, :, 1 : HWW + 1], tbov, op=MAX)

    # final med3(A, B, C) per column parity, written interleaved into OUT
    mn1 = t2("mn1", HWW); mx1 = t2("mx1", HWW); tf = t2("tf", HWW)
    mn1v, mx1v, tfv = tv(mn1, HWW), tv(mx1, HWW), tv(tf, HWW)
    for par, (Av, Bv, Cv) in ((0, (Aev, Bev, Cev)), (1, (Aov, Bov, Cov))):
        nc.vector.tensor_tensor(mn1v, Av, Bv, op=MIN)
        nc.vector.tensor_tensor(mx1v, Av, Bv, op=MAX)
        nc.vector.tensor_tensor(tfv, mx1v, Cv, op=MIN)
        nc.vector.tensor_tensor(ov[:, :, par:W:2], mn1v, tfv, op=MAX)


def _block(nc, pio, pm, xh, oh, g, half):
    """One vertical half (256 rows) of one image pair."""
    r0 = HW_half * half
    i0 = GIMG * g

    E = pio.tile([P, FW], F32, tag="E", name="E")
    O = pio.tile([P, FW], F32, tag="O", name="O")
    E_sh = pio.tile([P, FW], F32, tag="E_sh", name="E_sh")
    O_sh = pio.tile([P, FW], F32, tag="O_sh", name="O_sh")

    img = lambda r_lo, r_hi: xh[r_lo : min(r_hi, H) : 2, i0 : i0 + GIMG, :]
    # loads split across the two HWDGE queues (sync=SP, scalar=ACT)
    nc.sync.dma_start(E[:], img(r0, r0 + HW_half))
    nc.scalar.dma_start(O[:], img(r0 + 1, r0 + HW_half + 1))
    if half == 0:
        nc.sync.dma_start(E_sh[:], img(r0 + 2, r0 + HW_half + 2))
        nc.gpsimd.memset(O_sh[0:1, :], 0.0)
        nc.scalar.dma_start(O_sh[1:P, :], img(1, HW_half - 1))
    else:
        nc.gpsimd.memset(E_sh[96:P, :], 0.0)
        nc.sync.dma_start(E_sh[0 : P - 1, :], img(r0 + 2, H))
        nc.scalar.dma_start(O_sh[:], img(r0 - 1, r0 + HW_half - 1))

    # stage 1: vertical sort3 -> padded (min, med, max) per row parity
    pmn = pm.tile([P, FW], F32, tag="pmn", name="pmn")
    pmx = pm.tile([P, FW], F32, tag="pmx", name="pmx")
    nc.vector.tensor_tensor(pmn[:], E[:], O[:], op=MIN)
    nc.vector.tensor_tensor(pmx[:], E[:], O[:], op=MAX)

    padded = {}
    for name in ("MN_e", "MD_e", "MX_e", "MN_o", "MD_o", "MX_o"):
        T = pm.tile([P, FP], F32, tag=name, name=name)
        Tv = T[:].rearrange("p (i w) -> p i w", w=PW)
        # zero the two pad columns (0 and 513) of each image segment
        nc.vector.memset(Tv[:, :, 0 : PW : PW - 1], 0.0)
        padded[name] = T

    dv = lambda T: T[:].rearrange("p (i w) -> p i w", w=PW)[:, :, 1 : W + 1]
    wv = lambda T: T[:].rearrange("p (i w) -> p i w", w=W)
    # stage-1 temps alias the stage-2 A_e/A_o slots (disjoint lifetimes)
    t_o = pm.tile([P, FW], F32, tag="A_e", name="t_o")
    t_e = pm.tile([P, FW], F32, tag="A_o", name="t_e")

    # odd output rows (2p+1): pair + E_sh (row 2p+2)
    nc.vector.tensor_tensor(dv(padded["MN_o"]), wv(pmn), wv(E_sh), op=MIN)
    nc.vector.tensor_tensor(dv(padded["MX_o"]), wv(pmx), wv(E_sh), op=MAX)
    nc.vector.tensor_tensor(wv(t_o), wv(pmx), wv(E_sh), op=MIN)
    nc.vector.tensor_tensor(dv(padded["MD_o"]), wv(pmn), wv(t_o), op=MAX)
    # even output rows (2p): pair + O_sh (row 2p-1)
    nc.vector.tensor_tensor(dv(padded["MN_e"]), wv(pmn), wv(O_sh), op=MIN)
    nc.vector.tensor_tensor(dv(padded["MX_e"]), wv(pmx), wv(O_sh), op=MAX)
    nc.vector.tensor_tensor(wv(t_e), wv(pmx), wv(O_sh), op=MIN)
    nc.vector.tensor_tensor(dv(padded["MD_e"]), wv(pmn), wv(t_e), op=MAX)

    OUT_e = pio.tile([P, FW], F32, tag="OUT_e", name="OUT_e")
    OUT_o = pio.tile([P, FW], F32, tag="OUT_o", name="OUT_o")
    _stage2(nc, pm, padded["MN_e"], padded["MD_e"], padded["MX_e"], OUT_e, "e")
    _stage2(nc, pm, padded["MN_o"], padded["MD_o"], padded["MX_o"], OUT_o, "o")

    out_img = lambda r_lo, r_hi: oh[r_lo : min(r_hi, H) : 2, i0 : i0 + GIMG, :]
    nc.sync.dma_start(out_img(r0, r0 + HW_half), OUT_e[:])
    nc.scalar.dma_start(out_img(r0 + 1, r0 + HW_half + 1), OUT_o[:])


def build_program():
    nc = bacc.Bacc(
        "TRN2", target_bir_lowering=False, debug=False, num_devices=N_CORES
    )
    x_d = nc.dram_tensor("x", [B_PER, C, H, W], F32, kind="ExternalInput").ap()
    o_d = nc.dram_tensor("out", [B_PER, C, H, W], F32, kind="ExternalOutput").ap()
    xh = x_d.rearrange("b c h w -> h (b c) w")  # [512, 12, 512]
    oh = o_d.rearrange("b c h w -> h (b c) w")

    with tile.TileContext(nc) as tc:
        with (
            tc.tile_pool(name="io", bufs=1) as pio,
            tc.tile_pool(name="mid", bufs=1) as pm,
        ):
            for g in range(NIMG // GIMG):
                for half in range(2):
                    _block(nc, pio, pm, xh, oh, g, half)
    nc.compile()
    return nc


def _get_program():
    global _PROGRAM
    if _PROGRAM is None:
        _PROGRAM = build_program()
    return _PROGRAM


def kernel(**inputs) -> np.ndarray:
    x = np.ascontiguousarray(np.asarray(inputs["x"], dtype=np.float32))
    assert x.shape == (B, C, H, W), x.shape
    nc = _get_program()
    in_maps = [{"x": x[k * B_PER : (k + 1) * B_PER]} for k in range(N_CORES)]
    res = bass_utils.run_bass_kernel_spmd(nc, in_maps, core_ids=list(range(N_CORES)))
    return np.concatenate([res.results[k]["out"] for k in range(N_CORES)], axis=0)
